# revision 8
# baseline (speedup 1.0000x reference)
"""Cross-attention Trainium2 kernel (8-core data-parallel over batch).

Per-core computation (one batch element per NeuronCore):
  q = x @ Wq; k = ctx @ Wk; v = ctx @ Wv
  attn = softmax((q k^T) / sqrt(dh)); out = attn @ v; y = out @ Wo + bo

Everything on-chip is kept in "transposed" orientation (feature dim on
partitions, tokens on the free dim) so every matmul streams N=512-wide
moving operands:
  xT   [qd, tok]    via PE transposes of natural x tiles
  qT   [inner, tok] = Wq_chunk^T @ xT
  sT   [77, tok]    = k_hT^T @ q_hT          (head pairs packed in rows 0-63/64-127)
  e    [77, tok]    = exp(sT / 8)            (ACT; softmax max-subtraction not needed
                                              since |scores/8| <~ 6)
  r    [8, tok]     = column sums of e per head (selector matmul, PSUM-accumulated)
  outT [dh, tok]    = v_h^T @ e              (unnormalized)
  bc   [dh, tok]    = broadcast of 1/r_h     (selector matmul outer product)
  outT_norm         = outT * bc              (DVE, during PSUM->SBUF copy)
  y    [tok, qd]    = outT^T @ Wo + bo       (natural orientation, PSUM accum)
"""

import numpy as np

import concourse.bass as bass
import concourse.tile as tile
from concourse import bacc, mybir
from concourse.bass_utils import run_bass_kernel_spmd
from concourse.masks import make_identity

F32 = mybir.dt.float32

B, N, M = 8, 4096, 77
QD, CD, H, DH = 512, 768, 8, 64
INNER = H * DH  # 512
P = 128
S = 512  # token group size
NQC = QD // P  # 4 qd chunks
NCC = CD // P  # 6 cd chunks
NIC = INNER // P  # 4 inner chunks
NTS = S // P  # 4 token sub-tiles per group
SCALE = DH ** -0.5


def build_kernel(groups: int = N // S):
    nc = bacc.Bacc(None, target_bir_lowering=False, debug=False)

    x_d = nc.dram_tensor("x", [N, QD], F32, kind="ExternalInput")
    ctx_d = nc.dram_tensor("context", [M, CD], F32, kind="ExternalInput")
    wq_d = nc.dram_tensor("Wq", [QD, INNER], F32, kind="ExternalInput")
    wk_d = nc.dram_tensor("Wk", [CD, INNER], F32, kind="ExternalInput")
    wv_d = nc.dram_tensor("Wv", [CD, INNER], F32, kind="ExternalInput")
    wo_d = nc.dram_tensor("Wo", [INNER, QD], F32, kind="ExternalInput")
    bo_d = nc.dram_tensor("bo", [QD], F32, kind="ExternalInput")
    y_d = nc.dram_tensor("y", [N, QD], F32, kind="ExternalOutput")

    from contextlib import ExitStack

    with tile.TileContext(nc) as tc, ExitStack() as st:
        consts = st.enter_context(tc.tile_pool(name="consts", bufs=1))
        kvp = st.enter_context(tc.tile_pool(name="kv", bufs=1))
        xin = st.enter_context(tc.tile_pool(name="xin", bufs=3))
        xtp = st.enter_context(tc.tile_pool(name="xt", bufs=2))
        qtp = st.enter_context(tc.tile_pool(name="qt", bufs=2))
        expp = st.enter_context(tc.tile_pool(name="expp", bufs=2))
        rcp = st.enter_context(tc.tile_pool(name="rcp", bufs=2))
        outp = st.enter_context(tc.tile_pool(name="outp", bufs=2))
        yp = st.enter_context(tc.tile_pool(name="yp", bufs=2))

        # PSUM budget: 8 banks total.
        # ps_tr(2): PE-transpose outputs; ps_qf(2): q-proj + final-proj (+ k/v
        # setup) accumulators; ps_s(2): scores + recip-broadcast; ps_r(1):
        # rowsum accumulator; ps_o(1): attention output.
        ps_tr = st.enter_context(tc.tile_pool(name="ps_tr", bufs=2, space="PSUM"))
        ps_qf = st.enter_context(tc.tile_pool(name="ps_qf", bufs=2, space="PSUM"))
        ps_s = st.enter_context(tc.tile_pool(name="ps_s", bufs=2, space="PSUM"))
        ps_r = st.enter_context(tc.tile_pool(name="ps_r", bufs=1, space="PSUM"))
        ps_o = st.enter_context(tc.tile_pool(name="ps_o", bufs=1, space="PSUM"))

        # ---- constants / weights -------------------------------------------------
        identity = consts.tile([P, P], F32)
        make_identity(nc, identity)

        wq_sb = consts.tile([P, NQC, INNER], F32)
        nc.sync.dma_start(out=wq_sb, in_=wq_d.ap().rearrange("(c p) n -> p c n", p=P))
        wk_sb = consts.tile([P, NCC, INNER], F32)
        nc.sync.dma_start(out=wk_sb, in_=wk_d.ap().rearrange("(c p) n -> p c n", p=P))
        wv_sb = consts.tile([P, NCC, INNER], F32)
        nc.sync.dma_start(out=wv_sb, in_=wv_d.ap().rearrange("(c p) n -> p c n", p=P))
        wo_sb = consts.tile([P, NIC, QD], F32)
        nc.sync.dma_start(out=wo_sb, in_=wo_d.ap().rearrange("(c p) n -> p c n", p=P))

        bo_bc = consts.tile([P, QD], F32)
        bo_ap = bo_d.ap()
        nc.gpsimd.dma_start(
            out=bo_bc, in_=bass.AP(bo_ap.tensor, bo_ap.offset, [[0, P], [1, QD]])
        )

        # half-ones selectors: sel2[p, side, blk, j] = (blk == side).
        # lhsT = sel2[:, side] (= [77, 128] with ones in column block `side`)
        # makes a rowsum matmul write sum_p(exp_h[p, t]) replicated across
        # output partitions side*64 .. side*64+63 — i.e. the per-token softmax
        # denominator lands already broadcast across 64 partitions, two heads
        # per PSUM bank.
        sel2 = consts.tile([M, 2, 2, DH], F32)
        nc.gpsimd.memset(sel2, 0.0)
        nc.gpsimd.affine_select(
            out=sel2,
            in_=sel2,
            compare_op=mybir.AluOpType.not_equal,
            fill=1.0,
            base=0,
            # expr = -side + blk; == 0 where the 64-column block matches side
            pattern=[[-1, 2], [1, 2], [0, DH]],
            channel_multiplier=0,
        )

        # ---- context projections (tiny) -----------------------------------------
        ctx_sb = kvp.tile([M, CD], F32)
        nc.sync.dma_start(out=ctx_sb, in_=ctx_d[:, :])

        ctxT = kvp.tile([P, NCC, M], F32)
        for cc in range(NCC):
            pt = ps_tr.tile([P, P], F32, tag="ps_tr")
            nc.tensor.transpose(
                pt[:, :M], ctx_sb[:, cc * P : (cc + 1) * P], identity[:M, :M]
            )
            nc.vector.tensor_copy(out=ctxT[:, cc, :], in_=pt[:, :M])

        kT = kvp.tile([P, NIC, M], F32)
        for ic in range(NIC):
            pk = ps_qf.tile([P, S], F32, tag="ps_qf")
            for cc in range(NCC):
                nc.tensor.matmul(
                    pk[:, :M],
                    wk_sb[:, cc, ic * P : (ic + 1) * P],
                    ctxT[:, cc, :],
                    start=(cc == 0),
                    stop=(cc == NCC - 1),
                )
            nc.vector.tensor_copy(out=kT[:, ic, :], in_=pk[:, :M])

        v_sb = kvp.tile([M, INNER], F32)
        pv = ps_qf.tile([M, INNER], F32, tag="ps_qf")
        for cc in range(NCC):
            nc.tensor.matmul(
                pv,
                ctxT[:, cc, :],
                wv_sb[:, cc, :],
                start=(cc == 0),
                stop=(cc == NCC - 1),
            )
        nc.vector.tensor_copy(out=v_sb, in_=pv)

        # ---- main loop over token groups ----------------------------------------
        for g in range(groups):
            tok = slice(g * S, (g + 1) * S)

            x_g = xin.tile([P, NTS, QD], F32)
            nc.sync.dma_start(
                out=x_g, in_=x_d[tok, :].rearrange("(t p) q -> p t q", p=P)
            )

            # transpose x tiles: xT[p, c, t*128+j] = x[t*128+..., c*128+p]
            xT = xtp.tile([P, NQC, S], F32)
            for ts in range(NTS):
                for c in range(NQC):
                    pt = ps_tr.tile([P, P], F32, tag="ps_tr")
                    nc.tensor.transpose(
                        pt, x_g[:, ts, c * P : (c + 1) * P], identity
                    )
                    nc.vector.tensor_copy(
                        out=xT[:, c, ts * P : (ts + 1) * P], in_=pt
                    )

            # qT[inner, tok]
            qT = qtp.tile([P, NIC, S], F32)
            for ic in range(NIC):
                pq = ps_qf.tile([P, S], F32, tag="ps_qf")
                for c in range(NQC):
                    nc.tensor.matmul(
                        pq,
                        wq_sb[:, c, ic * P : (ic + 1) * P],
                        xT[:, c, :],
                        start=(c == 0),
                        stop=(c == NQC - 1),
                    )
                nc.vector.tensor_copy(out=qT[:, ic, :], in_=pq)

            # attention, phase 1: scores -> exp -> per-pair broadcast rowsums
            exp_g = expp.tile([M, H, S], F32)
            rec_g = rcp.tile([P, H // 2, S], F32)
            for h in range(H):
                ic, par = h // 2, (h % 2) * DH
                ps_sc = ps_s.tile([M, S], F32, tag="ps_s")
                nc.tensor.matmul(
                    ps_sc,
                    kT[par : par + DH, ic, :],
                    qT[par : par + DH, ic, :],
                    start=True,
                    stop=True,
                )
                nc.scalar.activation(
                    out=exp_g[:, h, :],
                    in_=ps_sc,
                    func=mybir.ActivationFunctionType.Exp,
                    scale=SCALE,
                )
                if h % 2 == 1:
                    pp = h // 2
                    pr = ps_r.tile([P, S], F32, tag="ps_r")
                    for side in range(2):
                        nc.tensor.matmul(
                            pr,
                            sel2[:, side],
                            exp_g[:, h - 1 + side, :],
                            start=(side == 0),
                            stop=(side == 1),
                        )
                    nc.vector.reciprocal(out=rec_g[:, pp, :], in_=pr)

            # attention, phase 2: outT (unnormalized) * (1/r) [already broadcast]
            outT = outp.tile([P, NIC, S], F32)
            for h in range(H):
                ic, par = h // 2, (h % 2) * DH
                po = ps_o.tile([DH, S], F32, tag="ps_o")
                nc.tensor.matmul(
                    po,
                    v_sb[:, h * DH : (h + 1) * DH],
                    exp_g[:, h, :],
                    start=True,
                    stop=True,
                )
                nc.vector.tensor_mul(
                    out=outT[par : par + DH, ic, :],
                    in0=po,
                    in1=rec_g[par : par + DH, h // 2, :],
                )

            # final projection + bias
            y_g = yp.tile([P, NTS, QD], F32)
            for ts in range(NTS):
                pf = ps_qf.tile([P, QD], F32, tag="ps_qf")
                for ic in range(NIC):
                    nc.tensor.matmul(
                        pf,
                        outT[:, ic, ts * P : (ts + 1) * P],
                        wo_sb[:, ic, :],
                        start=(ic == 0),
                        stop=(ic == NIC - 1),
                    )
                nc.vector.tensor_add(out=y_g[:, ts, :], in0=pf, in1=bo_bc)

            nc.sync.dma_start(
                out=y_d[tok, :].rearrange("(t p) q -> p t q", p=P), in_=y_g
            )

    nc.compile()
    return nc


_CACHE = {}


def _get_nc():
    if "nc" not in _CACHE:
        _CACHE["nc"] = build_kernel()
    return _CACHE["nc"]


def run(inputs, trace=False, **kw):
    nc = _get_nc()
    in_maps = []
    for i in range(B):
        m = {
            "x": np.asarray(inputs["x"][i], dtype=np.float32),
            "context": np.asarray(inputs["context"][i], dtype=np.float32),
            "Wq": np.asarray(inputs["Wq"], dtype=np.float32),
            "Wk": np.asarray(inputs["Wk"], dtype=np.float32),
            "Wv": np.asarray(inputs["Wv"], dtype=np.float32),
            "Wo": np.asarray(inputs["Wo"], dtype=np.float32),
            "bo": np.asarray(inputs["bo"], dtype=np.float32),
        }
        in_maps.append(m)
    res = run_bass_kernel_spmd(nc, in_maps, list(range(B)), trace=trace, **kw)
    out = np.stack([res.results[i]["y"] for i in range(B)], axis=0)
    return out, res


def kernel(**inputs):
    out, _ = run(inputs)
    return out
